# revision 14
# baseline (speedup 1.0000x reference)
"""Trainium2 Bass kernel for a 1-layer LSTM (T=512, B=256, IN=256, H=64)
followed by a sigmoid output projection to OUT=1024.

Strategy (data-parallel over batch, 8 cores x B_local=32):
  - Everything on-device is kept "transposed": hidden dim on partitions,
    (time x batch) on the free dimension.
  - All four gates use tanh via sigmoid(x) = (1 + tanh(x/2))/2.  The 0.5
    pre-scalings are folded into weights/biases (exact powers of two), and
    the states are kept doubled (ht = 2h, Ct = 2c) so the per-step cell
    update is 4 fused scalar_tensor_tensor DVE ops + 2 ACT tanh ops.
  - Gate pre-activations for a chunk of 8 timesteps live in one PSUM bank
    (128 partitions x [t0..t7] x [if | go] columns).  The x @ W_ih part plus
    bias is accumulated there ahead of time by bulk matmuls; the serial
    h @ W_hh part is added per step by two small accumulating matmuls.
  - The output projection runs as (65,128) x (65,1024) matmuls (ones row
    folds in b_out), sigmoid on ACT, interleaved into the recurrence's
    engine gaps.
"""

import numpy as np

import concourse.bass as bass
import concourse.bacc as bacc
import concourse.mybir as mybir
import concourse.tile as tile
from concourse.bass_utils import run_bass_kernel_spmd

F32 = mybir.dt.float32
AF = mybir.ActivationFunctionType
ALU = mybir.AluOpType

T_FULL = 512
B_FULL = 256
N_CORES = 8
BL = B_FULL // N_CORES  # 32
INPUT = 256
HID = 64
OUT = 1024


def build_lstm_nc(T=T_FULL, bl=BL, finalize=True):
    """Emit the single-core program (SPMD across 8 cores)."""
    nc = bacc.Bacc("TRN2", target_bir_lowering=False, debug=False)
    ntb = T * bl  # free-dim length of the (time x batch) axis

    # chunking
    CH = 8                      # timesteps per PSUM gate tile
    n_chunks = T // CH
    SLAB = 2048 if ntb % 2048 == 0 else ntb   # xT columns DMA'd at once
    n_slabs = ntb // SLAB
    chunks_per_slab = SLAB // (CH * bl)

    # ---- external I/O ----
    xT = nc.dram_tensor("xT", [INPUT, ntb], F32, kind="ExternalInput")
    wa = nc.dram_tensor("wa", [INPUT, 128], F32, kind="ExternalInput")
    wb = nc.dram_tensor("wb", [INPUT, 128], F32, kind="ExternalInput")
    wr1 = nc.dram_tensor("wr1", [HID, 128], F32, kind="ExternalInput")
    wr2 = nc.dram_tensor("wr2", [HID, 128], F32, kind="ExternalInput")
    b1 = nc.dram_tensor("b1", [1, 128], F32, kind="ExternalInput")
    b2 = nc.dram_tensor("b2", [1, 128], F32, kind="ExternalInput")
    wo = nc.dram_tensor("wo", [HID + 1, OUT], F32, kind="ExternalInput")
    h0 = nc.dram_tensor("h0", [HID, bl], F32, kind="ExternalInput")
    c0 = nc.dram_tensor("c0", [HID, bl], F32, kind="ExternalInput")

    out_d = nc.dram_tensor("out", [T, bl, OUT], F32, kind="ExternalOutput")
    hT_d = nc.dram_tensor("hT", [HID, bl], F32, kind="ExternalOutput")
    cT_d = nc.dram_tensor("cT", [HID, bl], F32, kind="ExternalOutput")

    from contextlib import ExitStack
    with tile.TileContext(nc) as tc, ExitStack() as ctx:
        const = ctx.enter_context(tc.tile_pool(name="const", bufs=1))
        slabs = ctx.enter_context(tc.tile_pool(name="slabs", bufs=2))
        psumG = ctx.enter_context(tc.tile_pool(name="psumG", bufs=2, space="PSUM"))
        psumO = ctx.enter_context(tc.tile_pool(name="psumO", bufs=4, space="PSUM"))
        hsp = ctx.enter_context(tc.tile_pool(name="hsp", bufs=4))
        gp = ctx.enter_context(tc.tile_pool(name="gp", bufs=3))
        cellp = ctx.enter_context(tc.tile_pool(name="cellp", bufs=3))
        obp = ctx.enter_context(tc.tile_pool(name="obp", bufs=3))

        # ---- constants into SBUF ----
        wa0_t = const.tile([128, 128], F32, tag="wa0")
        nc.sync.dma_start(wa0_t[:], wa[0:128, :])
        wa1_t = const.tile([128, 128], F32, tag="wa1")
        nc.sync.dma_start(wa1_t[:], wa[128:256, :])
        wb0_t = const.tile([128, 128], F32, tag="wb0")
        nc.sync.dma_start(wb0_t[:], wb[0:128, :])
        wb1_t = const.tile([128, 128], F32, tag="wb1")
        nc.sync.dma_start(wb1_t[:], wb[128:256, :])
        wr1_t = const.tile([HID, 128], F32, tag="wr1")
        nc.sync.dma_start(wr1_t[:], wr1[:])
        wr2_t = const.tile([HID, 128], F32, tag="wr2")
        nc.sync.dma_start(wr2_t[:], wr2[:])
        b1_t = const.tile([1, 128], F32, tag="b1")
        nc.sync.dma_start(b1_t[:], b1[:])
        b2_t = const.tile([1, 128], F32, tag="b2")
        nc.sync.dma_start(b2_t[:], b2[:])
        wo_t = const.tile([HID + 1, OUT], F32, tag="wo")
        nc.sync.dma_start(wo_t[:], wo[:])
        h0_t = const.tile([HID, bl], F32, tag="h0")
        nc.sync.dma_start(h0_t[:], h0[:])
        # cell state lives in partitions 64:128 so that fused DVE ops pair it
        # with the t_f / t_o gate slices (walrus requires equal base
        # partitions for both SBUF inputs of scalar_tensor_tensor)
        c0_t = const.tile([128, bl], F32, tag="c0")
        nc.sync.dma_start(c0_t[HID:128, :], c0[:])
        ones_t = const.tile([1, CH * bl], F32, tag="ones")
        nc.vector.memset(ones_t[:], 1.0)

        prev_h = h0_t[:]               # ht_{t-1}  (64, bl) AP, base 0
        prev_C = c0_t[HID:128, :]      # Ct_{t-1}  (64, bl) AP, base 64
        last_C = None

        # deferred output-projection work, spread across chain steps
        proj_tasks = []

        def emit_proj_task():
            if proj_tasks:
                proj_tasks.pop(0)()

        def make_proj(hsc, chunk_idx):
            # two halves of 4 timesteps -> (65,128) lhsT each
            def run(half, hsc=hsc, chunk_idx=chunk_idx):
                steps0 = chunk_idx * CH + half * 4
                lhsT = hsc[:, half * 4 * bl:(half + 1) * 4 * bl]
                po0 = psumO.tile([128, 512], F32, tag="po")
                po1 = psumO.tile([128, 512], F32, tag="po")
                ob = obp.tile([128, OUT], F32, tag="ob")

                def mm0():
                    nc.tensor.matmul(po0[:], lhsT, wo_t[:, 0:512],
                                     start=True, stop=True)

                def mm1():
                    nc.tensor.matmul(po1[:], lhsT, wo_t[:, 512:OUT],
                                     start=True, stop=True)

                def sig0():
                    nc.scalar.activation(ob[:, 0:512], po0[:], AF.Sigmoid)

                def sig1():
                    nc.scalar.activation(ob[:, 512:OUT], po1[:], AF.Sigmoid)

                def store():
                    dst = out_d[steps0:steps0 + 4].rearrange("t b o -> (t b) o")
                    nc.sync.dma_start(dst, ob[:])

                return [mm0, mm1, sig0, sig1, store]

            tasks = run(0) + run(1)
            proj_tasks.extend(tasks)

        for g in range(n_slabs):
            slabA = slabs.tile([128, SLAB], F32, tag="slabA")
            nc.sync.dma_start(slabA[:], xT[0:128, g * SLAB:(g + 1) * SLAB])
            slabB = slabs.tile([128, SLAB], F32, tag="slabB")
            nc.sync.dma_start(slabB[:], xT[128:256, g * SLAB:(g + 1) * SLAB])

            for ci in range(chunks_per_slab):
                chunk = g * chunks_per_slab + ci
                cols = slice(ci * CH * bl, (ci + 1) * CH * bl)

                # G columns: [0:256) = [i;f] block (col = 32t+b),
                #            [256:512) = [g;o] block (col = 256+32t+b)
                G = psumG.tile([128, CH * 64], F32, tag="G")
                half = CH * bl
                # (p, t, blk, b) strided view for the per-step activation
                G4 = G[:].rearrange("p (blk t b) -> p t blk b", blk=2, b=bl)

                nc.tensor.matmul(G[:, 0:half], wa0_t[:],
                                 slabA[:, cols], start=True, stop=False,
                                 skip_group_check=True)
                nc.tensor.matmul(G[:, 0:half], wa1_t[:],
                                 slabB[:, cols], start=False, stop=False,
                                 skip_group_check=True)
                nc.tensor.matmul(G[:, 0:half], b1_t[:], ones_t[:],
                                 start=False, stop=False,
                                 skip_group_check=True)
                # NOTE: start=True clears the whole bank, so only the very
                # first matmul of this tile uses it; first touch of the B
                # region (has_written=0) writes rather than accumulates.
                nc.tensor.matmul(G[:, half:2 * half], wb0_t[:],
                                 slabA[:, cols], start=False, stop=False,
                                 skip_group_check=True)
                nc.tensor.matmul(G[:, half:2 * half], wb1_t[:],
                                 slabB[:, cols], start=False, stop=False,
                                 skip_group_check=True)
                nc.tensor.matmul(G[:, half:2 * half], b2_t[:], ones_t[:],
                                 start=False, stop=False,
                                 skip_group_check=True)

                hsc = hsp.tile([HID + 1, CH * bl], F32, tag="hs")
                nc.vector.memset(hsc[HID:HID + 1, :], 1.0)

                for tl in range(CH):
                    acol = tl * bl
                    bcol = half + tl * bl
                    # serial part of the gates: += Wr.T @ ht_{t-1}
                    nc.tensor.matmul(G[:, acol:acol + bl], wr1_t[:], prev_h,
                                     start=False, stop=False,
                                     skip_group_check=True)
                    nc.tensor.matmul(G[:, bcol:bcol + bl], wr2_t[:],
                                     prev_h, start=False, stop=True,
                                     skip_group_check=True)

                    Tt = gp.tile([128, 64], F32, tag="T")
                    T3 = Tt[:].rearrange("p (blk b) -> p blk b", blk=2)
                    nc.scalar.activation(T3, G4[:, tl], AF.Tanh)

                    # w = (t_f + 1) * C ;  v = (t_i + 1) * gt ;
                    # C' = 0.5*w + v ;  th = tanh(0.5*C') ; ht = (t_o+1)*th
                    # (cell tiles use partitions 64:128 — see c0 note)
                    w_t = cellp.tile([128, bl], F32, tag="w")
                    nc.vector.scalar_tensor_tensor(
                        w_t[HID:128, :], Tt[HID:128, 0:bl], 1.0, prev_C,
                        ALU.add, ALU.mult)
                    v_t = cellp.tile([128, bl], F32, tag="v")
                    nc.vector.scalar_tensor_tensor(
                        v_t[HID:128, :], Tt[0:HID, 0:bl], 1.0,
                        Tt[0:HID, bl:64], ALU.add, ALU.mult)
                    C_t = cellp.tile([128, bl], F32, tag="C")
                    nc.vector.scalar_tensor_tensor(
                        C_t[HID:128, :], w_t[HID:128, :], 0.5,
                        v_t[HID:128, :], ALU.mult, ALU.add)
                    th_t = cellp.tile([128, bl], F32, tag="th")
                    nc.scalar.activation(th_t[HID:128, :], C_t[HID:128, :],
                                         AF.Tanh, scale=0.5)
                    hcol = tl * bl
                    nc.vector.scalar_tensor_tensor(
                        hsc[0:HID, hcol:hcol + bl], Tt[HID:128, bl:64], 1.0,
                        th_t[HID:128, :], ALU.add, ALU.mult)

                    prev_h = hsc[0:HID, hcol:hcol + bl]
                    prev_C = C_t[HID:128, :]
                    last_C = C_t

                    emit_proj_task()

                make_proj(hsc, chunk)

        while proj_tasks:
            emit_proj_task()

        nc.sync.dma_start(hT_d[:], prev_h)
        nc.sync.dma_start(cT_d[:], last_C[HID:128, :])

    if finalize:
        nc.finalize()
    return nc


# ---------------------------------------------------------------------------
# host side
# ---------------------------------------------------------------------------

def _prep_shared(W_ih, W_hh, b_ih, b_hh, W_out, b_out):
    W_ih = np.asarray(W_ih, np.float32)
    W_hh = np.asarray(W_hh, np.float32)
    bsum = (np.asarray(b_ih, np.float32) + np.asarray(b_hh, np.float32))
    W_out = np.asarray(W_out, np.float32)
    b_out = np.asarray(b_out, np.float32)

    # row scales: 0.5 for sigmoid gates (i, f, o), 1.0 for g
    s_if = 0.5
    s_go = np.concatenate([np.ones(HID), np.full(HID, 0.5)]).astype(np.float32)

    wa = np.ascontiguousarray((s_if * W_ih[0:128]).T)               # (256,128)
    wb = np.ascontiguousarray((s_go[:, None] * W_ih[128:256]).T)    # (256,128)
    # extra 0.5: chain matmul consumes ht = 2h
    wr1 = np.ascontiguousarray((0.5 * s_if * W_hh[0:128]).T)        # (64,128)
    wr2 = np.ascontiguousarray((0.5 * s_go[:, None] * W_hh[128:256]).T)
    b1 = np.ascontiguousarray((s_if * bsum[0:128])[None, :])        # (1,128)
    b2 = np.ascontiguousarray((s_go * bsum[128:256])[None, :])      # (1,128)
    wo = np.ascontiguousarray(
        np.concatenate([0.5 * W_out.T, b_out[None, :]], axis=0))    # (65,1024)
    return dict(wa=wa, wb=wb, wr1=wr1, wr2=wr2, b1=b1, b2=b2, wo=wo)


def make_in_maps(x, h, c, W_ih, W_hh, b_ih, b_hh, W_out, b_out,
                 T=T_FULL, n_cores=N_CORES):
    shared = _prep_shared(W_ih, W_hh, b_ih, b_hh, W_out, b_out)
    x = np.asarray(x, np.float32)
    h = np.asarray(h, np.float32)
    c = np.asarray(c, np.float32)
    in_maps = []
    for ci in range(n_cores):
        bs, be = ci * BL, (ci + 1) * BL
        xs = x[:T, bs:be, :]                       # (T, bl, IN)
        xT = np.ascontiguousarray(
            xs.transpose(2, 0, 1).reshape(INPUT, T * BL))
        h0 = np.ascontiguousarray(2.0 * h[0, bs:be, :].T)   # ht = 2h
        c0 = np.ascontiguousarray(2.0 * c[0, bs:be, :].T)   # Ct = 2c
        in_maps.append({"xT": xT, "h0": h0, "c0": c0, **shared})
    return in_maps


_NC_CACHE = {}


def _get_nc(T=T_FULL):
    if T not in _NC_CACHE:
        _NC_CACHE[T] = build_lstm_nc(T=T)
    return _NC_CACHE[T]


def kernel(x, h, c, W_ih, W_hh, b_ih, b_hh, W_out, b_out):
    nc = _get_nc()
    in_maps = make_in_maps(x, h, c, W_ih, W_hh, b_ih, b_hh, W_out, b_out)
    res = run_bass_kernel_spmd(nc, in_maps, core_ids=list(range(N_CORES)))
    outs = res.results
    out = np.concatenate([r["out"] for r in outs], axis=1)      # (T, B, OUT)
    hT = np.stack([0.5 * r["hT"].T for r in outs], axis=0)      # (8, bl, H)
    cT = np.stack([0.5 * r["cT"].T for r in outs], axis=0)
    hT = hT.reshape(1, B_FULL, HID)
    cT = cT.reshape(1, B_FULL, HID)
    return out, hT, cT
